# revision 20
# baseline (speedup 1.0000x reference)
"""CMPNN encoder on 8 Trainium2 NeuronCores (SPMD, bass/Tile). Self-contained.

kernel(**inputs) takes the FULL unsharded inputs and returns the FULL
[100000, 256] output. Host side: sort edges by dst, split into 8
run-aligned chunks; nodes range-sharded to match so segment_sum is
core-local. Per-core nodes are bin-packed into 128-slot blocks balanced
by degree, and edge slots are 128-tile-aligned per block region so the
scatter reads contiguous tiles (no indirect gather) with on-device
generated one-hot S tiles. Reverse-edge (e^1) values come from
duplicating the partner edge's state on the owning core. h_node is
replicated each iteration via Shared-output AllGather (bf16). All
matmul traffic is bf16; PSUM accumulation fp32.
"""
import os
import sys

sys.path.insert(0, "/opt/trn_rl_repo")

import heapq

import numpy as np

W = 8


# ---------------------------------------------------------------- drain patch
def _install_drain_patch():
    """This walrus build allows at most ONE sem wait per instruction; the
    Tile exit drain carries one per active proc domain. Re-emit the tail
    waits as single-wait nops on the sync queue before a bare drain."""
    import concourse.tile as tile
    from concourse import mybir
    from concourse.vector_clock import ScopedClock

    if getattr(tile.TileContext, "_drain_patched", False):
        return
    tile.TileContext._drain_patched = True

    def _drain_and_barrier(self, tick_clock, wait_clock):
        probe = self.nc.sync.nop(nofuse=True, hint="tile_exit_waits")
        wait_clock.add_sem_waits(
            probe.ins, ScopedClock({None: tick_clock.global_clock})
        )
        si = probe.ins.sync_info
        if si is not None and len(si.on_wait) > 1:
            waits = list(si.on_wait)
            probe.ins.sync_info = mybir.SyncInfo(
                on_wait=waits[:1], on_update=list(si.on_update)
            )
            for kk in range(1, len(waits)):
                extra = self.nc.sync.nop(nofuse=True, hint="tile_exit_waits")
                extra.ins.sync_info = mybir.SyncInfo(
                    on_wait=waits[kk:kk + 1], on_update=[]
                )
        self.nc.sync.drain()
        self.nc.all_engine_barrier()
        assert self.sems is not None
        popped = self.nc._tile_sem_poison_stack.pop()
        assert popped is self._sem_poison
        self.nc.clear_and_free_semaphores(list(self.sems.allocated().values()))
        self.nc.all_engine_barrier()

    tile.TileContext._drain_and_barrier = _drain_and_barrier


# ---------------------------------------------------------------- host prep

H = 256
A_DIM = 133
B_DIM = 147
DEPTH = 3          # message passing iterations
N_PAD = 12800      # local node slots (100 blocks of 128)
NBLK = N_PAD // 128
CHUNK_BLOCKS = [40, 30, 20, 10]  # allgather chunk sizes (blocks); small tail
N_CHUNKS = len(CHUNK_BLOCKS)
PAD_RANK = 127     # reserved pad node slot: block 0, rank 127
LDSTV_PAD = 300.0  # ldstv value marking a pad edge slot


def preprocess(node_attr, edge_attr, src, dst):
    """Shard + relayout. Returns per-core dicts and shared meta."""
    N, E = node_attr.shape[0], src.shape[0]
    order = np.argsort(dst, kind="stable").astype(np.int64)
    sdst = dst[order]
    # run-aligned chunk boundaries: ~E/W edges each, cut at dst-run boundary
    cuts = [0]
    for c in range(1, W):
        b = (E * c) // W
        while b < E and b > 0 and sdst[b] == sdst[b - 1]:
            b += 1
        cuts.append(b)
    cuts.append(E)
    node_lo = [0] * W
    node_hi = [0] * W
    for c in range(W):
        node_lo[c] = 0 if c == 0 else node_hi[c - 1]
        node_hi[c] = N if c == W - 1 else (sdst[cuts[c + 1]] if cuts[c + 1] < E else N)

    rev = np.arange(E, dtype=np.int64) ^ 1

    # per-core: balanced node->block assignment (cap 128/slot; block0 cap 127)
    cores = []
    e_cb_all = np.zeros((W, NBLK), np.int64)
    for c in range(W):
        lo, hi = cuts[c], cuts[c + 1]
        n_c = node_hi[c] - node_lo[c]
        assert n_c <= N_PAD - 1, f"core {c}: {n_c} nodes"
        ids = order[lo:hi]                       # owned edges (dst-sorted)
        ldst0 = (dst[ids] - node_lo[c]).astype(np.int64)   # local node id
        deg = np.bincount(ldst0, minlength=n_c)
        # greedy bin pack: big degrees first, into lightest non-full block
        nodes_by_deg = np.argsort(-deg, kind="stable")
        cap = np.full(NBLK, 128, np.int64)
        cap[0] = PAD_RANK                        # keep rank 127 of block 0 free
        heap = [(0, b) for b in range(NBLK)]
        heapq.heapify(heap)
        node_blk = np.empty(n_c, np.int64)
        cnt = np.zeros(NBLK, np.int64)
        esum = np.zeros(NBLK, np.int64)
        for n in nodes_by_deg:
            while True:
                s, b = heapq.heappop(heap)
                if cnt[b] < cap[b]:
                    break
            node_blk[n] = b
            cnt[b] += 1
            esum[b] += deg[n]
            if cnt[b] < cap[b]:
                heapq.heappush(heap, (esum[b], b))
        # ranks within block
        node_rank = np.empty(n_c, np.int64)
        for b in range(NBLK):
            sel = np.where(node_blk == b)[0]
            node_rank[sel] = np.arange(len(sel))
        e_cb_all[c] = esum
        cores.append({
            "lo": lo, "hi": hi, "ids": ids, "n_lo": node_lo[c], "n_hi": node_hi[c],
            "n_c": n_c, "ldst0": ldst0, "node_blk": node_blk, "node_rank": node_rank,
        })

    assert int(e_cb_all.max()) <= 512, f"block overflow: {e_cb_all.max()}"
    bcnt = np.full(NBLK, 4, np.int64)            # uniform: block b <-> group g
    t_tot = int(bcnt.sum())
    T0 = np.zeros(NBLK + 1, np.int64)
    np.cumsum(bcnt, out=T0[1:])
    e_pad2 = 128 * t_tot
    blk_of = np.repeat(np.arange(NBLK), bcnt)

    # global node -> replica index, chunk-major so each allgather chunk's
    # output is contiguous: rep[chunk][core][block - chunk_blk0][rank]
    cb0 = np.zeros(N_CHUNKS + 1, np.int64)       # chunk block starts
    np.cumsum(CHUNK_BLOCKS, out=cb0[1:])
    chunk_of_blk = np.searchsorted(cb0[1:], np.arange(NBLK), side="right")
    chunk_row0 = W * 128 * cb0                   # rep row offset of each chunk
    node_core = np.empty(N, np.int32)
    node_slot_local = np.empty(N, np.int64)      # block*128 + rank
    for c, d in enumerate(cores):
        g = np.arange(d["n_lo"], d["n_hi"])
        node_core[g] = c
        node_slot_local[g] = d["node_blk"] * 128 + d["node_rank"]
    _blk = node_slot_local // 128
    _ch = chunk_of_blk[_blk]
    _csz = 128 * np.asarray(CHUNK_BLOCKS, np.int64)[_ch]
    rep_idx = (chunk_row0[_ch] + node_core.astype(np.int64) * _csz
               + (_blk - cb0[_ch]) * 128 + (node_slot_local % 128))

    for c, d in enumerate(cores):
        ids = d["ids"]
        # edge slot: group owned edges by block of dst, tile-aligned regions
        eblk = d["node_blk"][d["ldst0"]]
        eorder = np.argsort(eblk, kind="stable")
        sids = ids[eorder]                       # edge ids in slot order
        sblk = eblk[eorder]
        # position within block run
        starts = np.searchsorted(sblk, np.arange(NBLK), side="left")
        pos = np.arange(len(sids)) - starts[sblk]
        slot = 128 * T0[sblk] + pos
        assert (pos < 128 * bcnt[sblk]).all()

        ea = np.zeros((e_pad2, B_DIM), np.float32)
        eap = np.zeros((e_pad2, B_DIM), np.float32)
        ea[slot] = edge_attr[sids]
        eap[slot] = edge_attr[rev[sids]]
        d["eaT"] = ea.T.copy()                   # [B, E_PAD2]
        d["eaT_p"] = eap.T.copy()

        pad_slot = c * CHUNK_BLOCKS[0] * 128 + PAD_RANK
        sidx = np.full(e_pad2, pad_slot, np.int64)
        sidx_p = np.full(e_pad2, pad_slot, np.int64)
        sidx[slot] = rep_idx[src[sids]]
        sidx_p[slot] = rep_idx[src[rev[sids]]]
        d["srcx"] = sidx.astype(np.int32).reshape(-1, 128).T.copy()    # [128, ET]
        d["srcx_p"] = sidx_p.astype(np.int32).reshape(-1, 128).T.copy()

        lv = np.full(e_pad2, LDSTV_PAD, np.float32)
        lv[slot] = d["node_rank"][d["ldst0"][eorder]].astype(np.float32)
        d["ldstv"] = lv.reshape(-1, 128).T.copy()                      # [128, t_tot]

        na = np.zeros((N_PAD, A_DIM), np.float32)
        loc = np.arange(d["n_lo"], d["n_hi"])
        na[node_slot_local[loc]] = node_attr[loc]
        d["naT"] = na.T.copy()                   # [A, N_PAD]
        d["slot_node"] = np.full(N_PAD, -1, np.int64)
        d["slot_node"][node_slot_local[loc]] = loc

    meta = {"bcnt": bcnt.tolist(), "t_tot": t_tot, "e_pad2": e_pad2,
            "T0": T0, "blk_of": blk_of, "cuts": cuts}
    return cores, meta


def mock_run(cores, meta, weights):
    """Numpy emulation of the device program (per-core SPMD + collectives)."""
    Wia, Wib, Wh, Wo, Wob, Wlr = (weights[k] for k in
                                  ("W_i_atom", "W_i_bond", "W_h", "W_o", "W_o_b", "W_lr"))
    relu = lambda x: np.maximum(x, 0)
    e_pad2 = meta["e_pad2"]
    t_tot = meta["t_tot"]
    blk_of = meta["blk_of"]
    st = []
    for d in cores:
        na = d["naT"].T
        h0n = relu(na @ Wia)                     # [N_PAD, H]
        h0e = relu(d["eaT"].T @ Wib)             # [E_PAD2, H]
        h0ep = relu(d["eaT_p"].T @ Wib)
        st.append({"h0n": h0n, "h0e": h0e, "h0ep": h0ep,
                   "he": h0e.copy(), "hep": h0ep.copy(), "hn": h0n.copy()})

    def scatter(s, d):
        agg = np.zeros((N_PAD, H), np.float32)
        lv = d["ldstv"].T.reshape(-1)            # [e_pad2] slot-order
        he = s["he"]
        for t in range(t_tot):
            b = blk_of[t]
            sl = slice(128 * t, 128 * (t + 1))
            cols = lv[sl]
            valid = cols < 128
            np.add.at(agg, b * 128 + cols[valid].astype(np.int64), he[sl][valid])
        return agg

    for it in range(DEPTH):
        for d, s in zip(cores, st):
            s["hn"] = s["hn"] + scatter(s, d)
        cb0 = np.zeros(N_CHUNKS + 1, np.int64)
        np.cumsum(CHUNK_BLOCKS, out=cb0[1:])
        rep = np.zeros((W * N_PAD, H), np.float32)
        for c, s in enumerate(st):
            for ch in range(N_CHUNKS):
                csz = 128 * CHUNK_BLOCKS[ch]
                base = W * 128 * cb0[ch]
                rep[base + c * csz: base + (c + 1) * csz] = \
                    s["hn"][128 * cb0[ch]:128 * cb0[ch + 1]]
        for d, s in zip(cores, st):
            sx = d["srcx"].T.reshape(-1)
            sxp = d["srcx_p"].T.reshape(-1)
            msg = rep[sx] - s["hep"]
            msg_p = rep[sxp] - s["he"]
            he_new = relu(s["h0e"] + msg @ Wh[it])
            hep_new = relu(s["h0ep"] + msg_p @ Wh[it])
            s["he"], s["hep"] = he_new, hep_new
    outs = []
    for d, s in zip(cores, st):
        agg = scatter(s, d)
        cat = np.concatenate([agg, s["hn"], s["h0n"]], axis=1)
        h = cat @ Wlr
        out = relu(np.concatenate([h, s["h0n"]], axis=1) @ Wo + Wob)
        outs.append(out)
    return outs


def unshard(outs, cores, N):
    out = np.zeros((N, H), np.float32)
    for d, o in zip(cores, outs):
        sel = d["slot_node"] >= 0
        out[d["slot_node"][sel]] = o[sel]
    return out


import concourse.tile as tile
from concourse import bacc, bass, mybir
from concourse.bass_utils import run_bass_kernel_spmd
from concourse.masks import make_identity

F32 = mybir.dt.float32
BF16 = mybir.dt.bfloat16
I32 = mybir.dt.int32


def build_bass(e_pad2, t_tot, bcnt, blk_of):
    _install_drain_patch()
    NG = N_PAD // 512          # node groups (epilogue)
    EG = e_pad2 // 512         # edge groups (prologue/update)
    ET = e_pad2 // 128         # edge tiles (== t_tot)
    assert ET == t_tot
    KB1 = B_DIM - 128          # second contraction chunk of bond dim (19)
    KA1 = A_DIM - 128          # second chunk of atom dim (5)
    nc = bacc.Bacc(None, num_devices=W)

    # ---- per-core inputs ----
    naT = nc.declare_dram_parameter("naT", [A_DIM, N_PAD], BF16, isOutput=False)
    eaT = nc.declare_dram_parameter("eaT", [256, e_pad2], BF16, isOutput=False)
    eaT_p = nc.declare_dram_parameter("eaT_p", [256, e_pad2], BF16, isOutput=False)
    srcx = nc.declare_dram_parameter("srcx", [128, ET], I32, isOutput=False)
    srcx_p = nc.declare_dram_parameter("srcx_p", [128, ET], I32, isOutput=False)
    ldstv = nc.declare_dram_parameter("ldstv", [128, t_tot], F32, isOutput=False)
    Wia = nc.declare_dram_parameter("Wia", [256, 256], BF16, isOutput=False)
    Wib = nc.declare_dram_parameter("Wib", [256, 256], BF16, isOutput=False)
    Whd = nc.declare_dram_parameter("Whd", [DEPTH, 256, 256], BF16, isOutput=False)
    Wlr = nc.declare_dram_parameter("Wlr", [768, 256], BF16, isOutput=False)
    Wo = nc.declare_dram_parameter("Wo", [512, 256], BF16, isOutput=False)
    Wob = nc.declare_dram_parameter("Wob", [256, 1], F32, isOutput=False)

    outT = nc.declare_dram_parameter("outT", [256, N_PAD], F32, isOutput=True)

    # ---- internal dram (states bf16) ----
    h0e = nc.dram_tensor("h0e", [e_pad2, H], BF16)
    h0ep = nc.dram_tensor("h0ep", [e_pad2, H], BF16)
    hea = nc.dram_tensor("hea", [e_pad2, H], BF16)
    heb = nc.dram_tensor("heb", [e_pad2, H], BF16)
    hepa = nc.dram_tensor("hepa", [e_pad2, H], BF16)
    hepb = nc.dram_tensor("hepb", [e_pad2, H], BF16)
    h0nT_d = nc.dram_tensor("h0nT_d", [2, 128, N_PAD], BF16)
    hn_row = nc.dram_tensor("hn_row", [N_PAD, H], BF16)
    ag_in = [nc.dram_tensor(f"ag_in{d}", [N_PAD, H], BF16) for d in range(DEPTH)]
    rep = [nc.dram_tensor(f"rep{d}", [W * N_PAD, H], BF16, addr_space="Shared")
           for d in range(DEPTH)]

    own_old = [h0e, hea, heb]     # read in iter d
    own_new = [hea, heb, hea]     # written in iter d
    par_old = [h0ep, hepa, hepb]
    par_new = [hepa, hepb, hepa]
    scat_src = [h0e, hea, heb, hea]  # edge rows read by scatter round it

    with tile.TileContext(nc) as tc:
        wpool = tc.alloc_tile_pool(name="w", bufs=1)
        resid = tc.alloc_tile_pool(name="resid", bufs=1)
        sb = tc.alloc_tile_pool(name="sb", bufs=2)
        sb2 = tc.alloc_tile_pool(name="sb2", bufs=2)
        sbg = tc.alloc_tile_pool(name="sbg", bufs=4)
        ps = tc.alloc_tile_pool(name="ps", bufs=2, space="PSUM")
        psr = tc.alloc_tile_pool(name="psr", bufs=2, space="PSUM")
        tp = tc.alloc_tile_pool(name="tp", bufs=4, space="PSUM")

        ident = wpool.tile([128, 128], BF16)
        make_identity(nc, ident[:])
        iota = wpool.tile([128, 128], F32)
        nc.gpsimd.iota(iota[:], pattern=[[1, 128]], base=0, channel_multiplier=0,
                       allow_small_or_imprecise_dtypes=True)

        def load_w(dram, kblocks, tag):
            t = wpool.tile([128, kblocks * 2 * 128], BF16, tag=tag)
            nc.sync.dma_start(
                t[:].rearrange("p (k o f) -> p k o f", k=kblocks, o=2),
                dram[:].rearrange("(k p) (o f) -> p k o f", p=128, o=2),
            )
            return t[:].rearrange("p (k o f) -> k o p f", k=kblocks, o=2), t

    # weight SBUF layout: view[kb][ob] -> [128K, 128M]; raw[:, kb*256:(kb+1)*256]
    # is the contiguous [128K, 256M] moving block for stationary-data matmuls.
        wia, wia_raw = load_w(Wia, 2, "wia")
        wib, wib_raw = load_w(Wib, 2, "wib")
        whd_raw = [load_w(Whd[d], 2, f"whd{d}")[1] for d in range(DEPTH)]
        wlr, _ = load_w(Wlr, 6, "wlr")
        wo, _ = load_w(Wo, 4, "wo")
        wob = wpool.tile([128, 2], F32)
        nc.sync.dma_start(wob[:], Wob[:].rearrange("(o p) c -> p (o c)", p=128))

        srcx_sb = resid.tile([128, ET], I32)
        srcxp_sb = resid.tile([128, ET], I32)
        ldstv_sb = resid.tile([128, t_tot], F32)
        nc.sync.dma_start(srcx_sb[:], srcx[:])
        nc.sync.dma_start(srcxp_sb[:], srcx_p[:])
        nc.sync.dma_start(ldstv_sb[:], ldstv[:])

        # resident node state [128, NB*256]; block b at cols b*H..(b+1)*H
        hn = resid.tile([128, NBLK * H], BF16, tag="hnagg")

        def proj_512(wt, xT_dram, kdims, col0, relu_out):
            """relu(W^T @ xT[:, col0:col0+512]) -> 2 bf16 fm tiles [128,512]."""
            xsb = sb2.tile([128, 2 * 512], BF16, tag="msgT")
            nc.sync.dma_start(
                xsb[:kdims[0], 0:512], xT_dram[0:kdims[0], col0:col0 + 512]
            )
            if kdims[1]:
                nc.sync.dma_start(
                    xsb[:kdims[1], 512:1024],
                    xT_dram[128:128 + kdims[1], col0:col0 + 512],
                )
            outs = []
            for ob in range(2):
                p = ps.tile([128, 512], F32, tag="mm")
                for kb in range(2):
                    if kdims[kb] == 0:
                        continue
                    nc.tensor.matmul(
                        p[:],
                        wt[kb, ob][:kdims[kb], :],
                        xsb[:kdims[kb], kb * 512:kb * 512 + 512],
                        start=(kb == 0),
                        stop=(kb == 1 or kdims[1] == 0),
                    )
                o = sb.tile([128, 512], BF16, tag="proj_o")
                nc.scalar.activation(o[:], p[:], mybir.ActivationFunctionType.Relu)
                outs.append(o)
                if relu_out is not None:
                    nc.sync.dma_start(relu_out[ob, :, col0:col0 + 512], o[:])
            return outs

        def transpose_fm_to_rows(fm_tiles, row_sb, eng):
            for ob in range(2):
                for j in range(4):
                    pt = tp.tile([128, 128], BF16, tag="tp")
                    nc.tensor.transpose(
                        pt[:], fm_tiles[ob][:, j * 128:(j + 1) * 128], ident[:]
                    )
                    eng(j)(
                        row_sb[:, j * H + ob * 128: j * H + ob * 128 + 128], pt[:]
                    )

        def eng_alt(j):
            return nc.vector.tensor_copy if j % 2 == 0 else nc.scalar.copy

        # ---------------- prologue: edges (data-stationary, row-major out) ----
        def edge_prologue(xT_dram, out_dram, g):
            col0 = g * 512
            xa = sb2.tile([128, 2 * 512], BF16, tag="msgT")
            nc.sync.dma_start(
                xa[:].rearrange("p (k e) -> p k e", k=2),
                xT_dram[:, col0:col0 + 512].rearrange("(k p) e -> p k e", p=128),
            )
            rows = sb2.tile([128, 4 * H], BF16, tag="upd_rows")
            for jj in range(2):
                pr = psr.tile([128, 512], F32, tag="rw2")
                for hh in range(2):
                    j = jj * 2 + hh
                    nc.tensor.matmul(
                        pr[:, hh * H:(hh + 1) * H],
                        xa[:, j * 128:(j + 1) * 128], wib_raw[:, 0:256],
                        start=True, stop=False,
                    )
                    nc.tensor.matmul(
                        pr[:, hh * H:(hh + 1) * H],
                        xa[:KB1, 512 + j * 128:512 + (j + 1) * 128],
                        wib_raw[:KB1, 256:512],
                        start=False, stop=True,
                    )
                nc.scalar.activation(
                    rows[:, jj * 512:(jj + 1) * 512], pr[:],
                    mybir.ActivationFunctionType.Relu,
                )
            nc.sync.dma_start(
                out_dram[col0:col0 + 512, :].rearrange("(j p) f -> p j f", p=128),
                rows[:].rearrange("p (j f) -> p j f", j=4),
            )
            return rows

        # ---------------- prologue: nodes (one 4-block group) -------------
        def node_prologue(g):
            col0 = g * 512
            n_fm = proj_512(wia, naT, (128, KA1), col0, h0nT_d)
            rows = sb2.tile([128, 4 * H], BF16, tag="upd_rows")
            transpose_fm_to_rows(n_fm, rows, eng_alt)
            for j in range(4):
                nc.gpsimd.tensor_copy(
                    hn[:, (g * 4 + j) * H:(g * 4 + j + 1) * H],
                    rows[:, j * H:(j + 1) * H],
                )

        # ---------------- scatter: SBUF-direct tiles + on-device S ---------
        CB0 = [0]
        for nbl in CHUNK_BLOCKS:
            CB0.append(CB0[-1] + nbl)

        def ag_chunk(d, ch):
            """DMA hn chunk to ag_in and allgather it into rep[d] (contiguous)."""
            lo, hi = CB0[ch] * 128, CB0[ch + 1] * 128
            nc.sync.dma_start(
                ag_in[d][lo:hi, :].rearrange("(b p) f -> p b f", p=128),
                hn[:, CB0[ch] * H:CB0[ch + 1] * H].rearrange(
                    "p (b f) -> p b f", f=256),
            )
            nc.gpsimd.collective_compute(
                "AllGather",
                mybir.AluOpType.bypass,
                replica_groups=[list(range(W))],
                ins=[ag_in[d][lo:hi, :].opt()],
                outs=[rep[d][W * lo:W * hi, :].opt()],
            )

        def scatter_block(target, b, rows):
            """Scatter block b straight from the SBUF rows tile of the group
            that just produced its 4 tiles (no DRAM round-trip)."""
            p = psr.tile([128, 512], F32, tag="rw2")
            for kk in range(4):
                t = 4 * b + kk
                s_sb = sb.tile([128, 128], BF16, tag="scat_S")
                nc.vector.tensor_scalar(
                    s_sb[:], iota[:], ldstv_sb[:, t:t + 1], None,
                    op0=mybir.AluOpType.is_equal,
                )
                nc.tensor.matmul(
                    p[:, 0:H], s_sb[:], rows[:, kk * H:(kk + 1) * H],
                    start=(kk == 0), stop=(kk == 3),
                )
            dst = target[:, b * H:(b + 1) * H]
            nc.vector.tensor_add(dst, dst, p[:, 0:H])

        # ---------------- update (gather + transpose + data-stationary mm) --
        def update_stream(d, idx_sb, sub_rows_dram, h0_dram, out_dram, g):
            col0 = g * 512
            gat = sbg.tile([128, 4 * H], BF16, tag="gat")
            for k in range(4):
                nc.gpsimd.indirect_dma_start(
                    out=gat[:, k * H:(k + 1) * H],
                    out_offset=None,
                    in_=rep[d][:],
                    in_offset=bass.IndirectOffsetOnAxis(
                        ap=idx_sb[:, g * 4 + k:g * 4 + k + 1], axis=0
                    ),
                )
            old = sb2.tile([128, 4 * H], BF16, tag="old")
            nc.sync.dma_start(
                old[:].rearrange("p (j f) -> p j f", j=4),
                sub_rows_dram[col0:col0 + 512, :].rearrange("(j p) f -> p j f", p=128),
            )
            msg = sb2.tile([128, 4 * H], BF16, tag="msg")
            nc.vector.tensor_sub(msg[:], gat[:], old[:])
            # transpose msg -> feature-major msgT [hi, (kb: 512 edges)]
            msgT = sb2.tile([128, 2 * 512], BF16, tag="msgT")
            for j in range(4):
                for ob in range(2):
                    pt = tp.tile([128, 128], BF16, tag="tp")
                    nc.tensor.transpose(
                        pt[:], msg[:, j * H + ob * 128:j * H + ob * 128 + 128],
                        ident[:],
                    )
                    eng = (nc.vector.tensor_copy if (j * 2 + ob) % 4 == 0
                           else nc.scalar.copy)
                    eng(msgT[:, ob * 512 + j * 128:ob * 512 + j * 128 + 128],
                        pt[:])
            h0sb = sb2.tile([128, 4 * H], BF16, tag="h0sb")
            nc.sync.dma_start(
                h0sb[:].rearrange("p (j f) -> p j f", j=4),
                h0_dram[col0:col0 + 512, :].rearrange("(j p) f -> p j f", p=128),
            )
            rows = sb2.tile([128, 4 * H], BF16, tag="upd_rows")
            wt_raw = whd_raw[d]
            for jj in range(2):
                pr = psr.tile([128, 512], F32, tag="rw2")
                for hh in range(2):
                    j = jj * 2 + hh
                    for kb in range(2):
                        nc.tensor.matmul(
                            pr[:, hh * H:(hh + 1) * H],
                            msgT[:, kb * 512 + j * 128:kb * 512 + (j + 1) * 128],
                            wt_raw[:, kb * 256:(kb + 1) * 256],
                            start=(kb == 0), stop=(kb == 1),
                        )
                nc.vector.tensor_add(
                    rows[:, jj * 512:(jj + 1) * 512], pr[:],
                    h0sb[:, jj * 512:(jj + 1) * 512]
                )
            nc.scalar.activation(rows[:], rows[:], mybir.ActivationFunctionType.Relu)
            nc.sync.dma_start(
                out_dram[col0:col0 + 512, :].rearrange("(j p) f -> p j f", p=128),
                rows[:].rearrange("p (j f) -> p j f", j=4),
            )
            return rows

        # ---------------- epilogue (callable, interleaved into iter 2) ----
        def epilogue_group(g):
            col0 = g * 512
            catT = sb2.tile([128, 6 * 512], BF16, tag="gat")
            for j in range(4):
                b = g * 4 + j
                for ob in range(2):
                    pt = tp.tile([128, 128], BF16, tag="tp")
                    nc.tensor.transpose(
                        pt[:], agg[:, b * H + ob * 128:b * H + ob * 128 + 128],
                        ident[:],
                    )
                    eng_alt(j)(catT[:, ob * 512 + j * 128:ob * 512 + j * 128 + 128],
                               pt[:])
            hrow = sb2.tile([128, 4 * H], BF16, tag="h0sb")
            nc.sync.dma_start(
                hrow[:].rearrange("p (j f) -> p j f", j=4),
                hn_row[col0:col0 + 512, :].rearrange("(j p) f -> p j f", p=128),
            )
            for j in range(4):
                for ob in range(2):
                    pt = tp.tile([128, 128], BF16, tag="tp")
                    nc.tensor.transpose(
                        pt[:], hrow[:, j * H + ob * 128:j * H + ob * 128 + 128],
                        ident[:],
                    )
                    eng_alt(j)(
                        catT[:, (2 + ob) * 512 + j * 128:(2 + ob) * 512 + j * 128 + 128],
                        pt[:],
                    )
            nc.sync.dma_start(
                catT[:, 4 * 512:6 * 512].rearrange("p (o f) -> p o f", o=2),
                h0nT_d[:, :, col0:col0 + 512].rearrange("o p f -> p o f"),
            )
            hT = sb2.tile([128, 2 * 512], BF16, tag="old")
            for ob in range(2):
                p = ps.tile([128, 512], F32, tag="mm")
                for kb in range(6):
                    nc.tensor.matmul(
                        p[:], wlr[kb, ob], catT[:, kb * 512:kb * 512 + 512],
                        start=(kb == 0), stop=(kb == 5),
                    )
                nc.vector.tensor_copy(hT[:, ob * 512:(ob + 1) * 512], p[:])
            for ob in range(2):
                p = ps.tile([128, 512], F32, tag="mm")
                for kb in range(4):
                    rhs = (hT[:, (kb % 2) * 512:(kb % 2) * 512 + 512] if kb < 2
                           else catT[:, (4 + kb % 2) * 512:(4 + kb % 2) * 512 + 512])
                    nc.tensor.matmul(
                        p[:], wo[kb, ob], rhs,
                        start=(kb == 0), stop=(kb == 3),
                    )
                o = sb.tile([128, 512], F32, tag="out_o")
                nc.scalar.activation(
                    o[:], p[:], mybir.ActivationFunctionType.Relu,
                    bias=wob[:, ob:ob + 1],
                )
                nc.sync.dma_start(outT[ob * 128:(ob + 1) * 128, col0:col0 + 512], o[:])


        def maybe_ag(d, b):
            """Fire the allgather chunk whose last block just completed."""
            if (b + 1) in CB0[1:]:
                ag_chunk(d, CB0.index(b + 1) - 1)

        # ------- prologue (nodes+edges both streams) + scatter0 + CC0 -----
        # single dense phase: keeps the PE ramped and the CC0 chain fully in
        # the shadow of the partner-stream projection
        for g in range(EG):
            if g % 4 == 0:
                node_prologue(g // 4)
            rows = edge_prologue(eaT, h0e, g)
            scatter_block(hn, g, rows)
            edge_prologue(eaT_p, h0ep, g)
            maybe_ag(0, g)

        # ---------------- iterations (next scatter+CC run inside updates) --
        agg = None
        ep_done = 0
        for d in range(DEPTH):
            last = d == DEPTH - 1
            if last:
                # hn fully consumed (last ag chunks fired inside iter d-1);
                # dump rows for the epilogue and reuse the buffer as agg
                nc.sync.dma_start(
                    hn_row[:].rearrange("(b p) f -> p b f", p=128),
                    hn[:].rearrange("p (b f) -> p b f", f=256),
                )
                agg = resid.tile([128, NBLK * H], BF16, tag="hnagg")
                nc.vector.memset(agg[:], 0.0)
            for g in range(EG):
                rows = update_stream(d, srcx_sb, par_old[d], h0e, own_new[d], g)
                update_stream(d, srcxp_sb, own_old[d], h0ep, par_new[d], g)
                if not last:
                    scatter_block(hn, g, rows)
                    maybe_ag(d + 1, g)
                else:
                    scatter_block(agg, g, rows)
                    while (ep_done + 1) * 4 <= g + 1 - 3:
                        epilogue_group(ep_done)
                        ep_done += 1
        while ep_done < NG:
            epilogue_group(ep_done)
            ep_done += 1

        tp.release()
        psr.release()
        sbg.release()
        ps.release()
        sb2.release()
        sb.release()
        resid.release()
        wpool.release()
    nc.finalize()
    return nc


def build_and_maps(inputs, cores, meta):
    """Build the Bass program and per-core input maps from preprocessed shards."""
    import ml_dtypes
    bf16 = ml_dtypes.bfloat16
    W_i_atom = inputs["W_i_atom"]; W_i_bond = inputs["W_i_bond"]
    W_h = inputs["W_h"]; W_o = inputs["W_o"]; W_o_b = inputs["W_o_b"]; W_lr = inputs["W_lr"]

    def padk(w, k):
        w = np.asarray(w, np.float32)
        return np.pad(w, ((0, k - w.shape[0]), (0, 0)))

    wts = {
        "Wia": padk(W_i_atom, 256).astype(bf16),
        "Wib": padk(W_i_bond, 256).astype(bf16),
        "Whd": np.asarray(W_h, np.float32).astype(bf16),
        "Wlr": np.asarray(W_lr, np.float32).astype(bf16),
        "Wo": np.asarray(W_o, np.float32).astype(bf16),
        "Wob": np.asarray(W_o_b, np.float32).reshape(256, 1),
    }
    in_maps = []
    for d in cores:
        m = dict(wts)
        m["naT"] = d["naT"].astype(bf16)
        m["eaT"] = np.pad(d["eaT"], ((0, 256 - B_DIM), (0, 0))).astype(bf16)
        m["eaT_p"] = np.pad(d["eaT_p"], ((0, 256 - B_DIM), (0, 0))).astype(bf16)
        m["srcx"] = d["srcx"]
        m["srcx_p"] = d["srcx_p"]
        m["ldstv"] = d["ldstv"]
        in_maps.append(m)
    nc = build_bass(meta["e_pad2"], meta["t_tot"], meta["bcnt"], meta["blk_of"])
    return nc, in_maps


def kernel(node_attr, edge_attr, W_i_atom, W_i_bond, W_h, W_o, W_o_b, W_lr, src, dst):
    node_attr = np.asarray(node_attr, np.float32)
    edge_attr = np.asarray(edge_attr, np.float32)
    src = np.asarray(src, np.int32)
    dst = np.asarray(dst, np.int32)
    cores, meta = preprocess(node_attr, edge_attr, src, dst)
    inputs = {"W_i_atom": W_i_atom, "W_i_bond": W_i_bond, "W_h": W_h,
              "W_o": W_o, "W_o_b": W_o_b, "W_lr": W_lr}
    nc, in_maps = build_and_maps(inputs, cores, meta)

    res = None
    for attempt in range(3):
        try:
            res = run_bass_kernel_spmd(nc, in_maps, core_ids=list(range(W)))
            break
        except Exception:
            if attempt == 2:
                raise
            import time as _time
            _time.sleep(5)
    outs = [np.asarray(r["outT"], np.float32).T for r in res.results]  # [N_PAD, H]
    return unshard(outs, cores, node_attr.shape[0])
